# revision 34
# baseline (speedup 1.0000x reference)
"""Bass/Tile kernel for a 4-layer dense transformer (prefill) on 8 TRN2 cores.

Parallelization: 2-way data parallel (batch) x 4-way tensor parallel with a
sequence-parallel residual stream (Megatron-SP style).
Groups: cores [0,1,2,3] handle batch 0, [4,5,6,7] batch 1.
Within a group (rank r = core % 4):
  - residual stream x: OWN tokens r*256..(r+1)*256, transposed [D, 256] f32
  - layernorms computed on the 256 own tokens only
  - attention: heads r*4..r*4+3 over all 1024 tokens (h1 AllGathered)
  - attn out projection: row-parallel Wo partial over all tokens;
    ReduceScatter (bf16) delivers each rank the full-D delta for its tokens
  - MLP: fully LOCAL on own tokens with full W1/W2 streamed from HBM
    (no collective at all; weights are replicated instead of sharded)
  - vocab: cols r*8000..(r+1)*8000 of head_w for the logits
Collectives per layer: AG(h1 bf16 0.5MB) + RS(d1 bf16 2MB) only.
PE keep-warm dummy matmul chains run during the collective stalls so the
HAM clock gate stays at 2.4 GHz.
Final logits are computed in natural [token, vocab] layout and written out
per-core as [1024, 8000]; the host concatenates.
"""

import sys
import types

import numpy as np


def _install_ntff_shim():
    """Register the NTFF profiling hook that trn_boot skipped (the image's
    antenv package lacks the axon_hooks submodule)."""
    if "antenv.axon_hooks" in sys.modules:
        return
    try:
        import trn_agent_boot.trn_boot as tb
        hook = tb._ntff_profile_via_ctypes("/opt/axon/libaxon_pjrt.so")
    except Exception:
        hook = None
    mod = types.ModuleType("antenv.axon_hooks")
    _h = [hook]
    mod.get_axon_ntff_profile_hook = lambda: _h[0]
    mod.set_axon_ntff_profile_hook = lambda h: _h.__setitem__(0, h)
    sys.modules["antenv.axon_hooks"] = mod
    try:
        import antenv
        antenv.axon_hooks = mod
    except Exception:
        pass


_install_ntff_shim()

import ml_dtypes
import concourse.bass as bass
import concourse.mybir as mybir
import concourse.tile as tile
from concourse import bacc
from concourse.bass_utils import run_bass_kernel_spmd

BF = mybir.dt.bfloat16
F32 = mybir.dt.float32
AL = mybir.AluOpType
AF = mybir.ActivationFunctionType

# Model sizes (full problem, hardcoded per contract).
CFG = dict(
    B=2, S=1024, V=32000, D=1024, H=16, L=4, EPS=1e-5,
    TP=4,            # tensor-parallel width (group size)
    gelu_sim=False,  # CoreSim lacks Gelu; use sigmoid-based stand-in
    warm_ag=90,      # keep-warm matmuls during AG(h1)
    warm_rs=150,     # keep-warm matmuls during RS(d1)
    warm_f=100,      # keep-warm matmuls during final AG
)

N_CORES = 8
GROUPS = [[0, 1, 2, 3], [4, 5, 6, 7]]


def build_program(cfg=None):
    """Build the SPMD Bass program (identical on all 8 cores)."""
    c = dict(CFG)
    if cfg:
        c.update(cfg)
    B, S, V, D, H, L = c["B"], c["S"], c["V"], c["D"], c["H"], c["L"]
    EPS, TP = c["EPS"], c["TP"]
    T = S                    # tokens per group (one batch element)
    TS = T // TP             # own tokens per rank (256)
    DK = D // H              # head dim (64)
    HL = H // TP             # heads per core (4)
    DSH = D // TP            # attention feature shard (256)
    DF = 4 * D
    VSH = V // TP            # vocab shard (8000)
    KT = D // 128            # feature k-tiles (8)
    KF = DF // 128           # full mlp hidden tiles (32)
    NCH = max(1, T // 512)   # token chunks of <=512
    TCH = min(512, T)        # token chunk size
    MSH = DSH // 128         # m-tiles of a DSH-wide slab (2)
    TT = T // 128            # token tiles (8)
    VCH = 500                # vocab chunk
    NV = VSH // VCH          # vocab n-chunks (16)
    MW = 4                   # w1 m-tiles per streamed chunk
    assert T % 128 == 0 and D % 128 == 0 and DSH % 128 == 0 and TS % 128 == 0

    groups = [[g * TP + r for r in range(TP)] for g in range(N_CORES // TP)]

    nc = bacc.Bacc("TRN2", target_bir_lowering=False, debug=False,
                   num_devices=N_CORES)

    # ---- DRAM parameters (per-core shards fed via in_maps, pre-tiled) ----
    xT0 = nc.dram_tensor("xT0", [D, TS], F32, kind="ExternalInput")
    wq = nc.dram_tensor("wq", [L, 128, KT * DSH], BF, kind="ExternalInput")
    wk = nc.dram_tensor("wk", [L, 128, KT * DSH], BF, kind="ExternalInput")
    wv = nc.dram_tensor("wv", [L, 128, KT * DSH], BF, kind="ExternalInput")
    wo = nc.dram_tensor("wo", [L, 128, MSH * D], BF, kind="ExternalInput")
    w1 = nc.dram_tensor("w1", [L, 128, KT * DF], BF, kind="ExternalInput")
    w2 = nc.dram_tensor("w2", [L, 128, KT, KF * 128], BF,
                        kind="ExternalInput")
    b1 = nc.dram_tensor("b1", [L, 128, KF], F32, kind="ExternalInput")
    b2 = nc.dram_tensor("b2", [L, 128, KT], F32, kind="ExternalInput")
    g1 = nc.dram_tensor("g1", [L, 128, KT], F32, kind="ExternalInput")
    be1 = nc.dram_tensor("be1", [L, 128, KT], F32, kind="ExternalInput")
    g2 = nc.dram_tensor("g2", [L, 128, KT], F32, kind="ExternalInput")
    be2 = nc.dram_tensor("be2", [L, 128, KT], F32, kind="ExternalInput")
    gf = nc.dram_tensor("gf", [128, KT], F32, kind="ExternalInput")
    bef = nc.dram_tensor("bef", [128, KT], F32, kind="ExternalInput")
    hw = nc.dram_tensor("hw", [128, NV, KT, VCH], BF, kind="ExternalInput")
    logits = nc.dram_tensor("logits", [T, VSH], F32, kind="ExternalOutput")
    dbg = {}
    if c.get("dbg"):
        dbg["h1"] = nc.dram_tensor("dbg_h1", [D, T], BF, kind="ExternalOutput")
        dbg["o"] = nc.dram_tensor("dbg_o", [DSH, T], BF, kind="ExternalOutput")
        dbg["x1"] = nc.dram_tensor("dbg_x1", [D, TS], F32,
                                   kind="ExternalOutput")
        dbg["x2"] = nc.dram_tensor("dbg_x2", [D, TS], F32,
                                   kind="ExternalOutput")

    with tile.TileContext(nc) as tc:
        _build_tc(nc, tc, locals())
    nc.compile()
    return nc


def _build_tc(nc, tc, v):
    """Emit the tile program. `v` is the name->value dict from build_program."""
    (B, T, TS, D, L, EPS, TP, DK, HL, DSH, DF, VSH, KT, KF, NCH, TCH,
     MSH, NV, VCH, TT, MW, groups) = (
        v["B"], v["T"], v["TS"], v["D"], v["L"], v["EPS"], v["TP"], v["DK"],
        v["HL"], v["DSH"], v["DF"], v["VSH"], v["KT"], v["KF"], v["NCH"],
        v["TCH"], v["MSH"], v["NV"], v["VCH"], v["TT"], v["MW"], v["groups"])
    xT0, wq, wk, wv, wo, w1, w2 = (v["xT0"], v["wq"], v["wk"], v["wv"],
                                   v["wo"], v["w1"], v["w2"])
    b1d, b2d, g1d, be1d, g2d, be2d, gfd, befd = (
        v["b1"], v["b2"], v["g1"], v["be1"], v["g2"], v["be2"], v["gf"],
        v["bef"])
    hwd, logits, dbg = v["hw"], v["logits"], v["dbg"]
    cfgc = v["c"]

    import contextlib
    ctx = contextlib.ExitStack()

    # ---------------- pools ----------------
    sing = ctx.enter_context(tc.tile_pool(name="sing", bufs=1))
    wts = ctx.enter_context(tc.tile_pool(name="wts", bufs=1))
    w1p = ctx.enter_context(tc.tile_pool(name="w1p", bufs=3))   # w1 chunks
    w2p = ctx.enter_context(tc.tile_pool(name="w2p", bufs=3))   # w2 m-tiles
    tiny = ctx.enter_context(tc.tile_pool(name="tiny", bufs=2))
    hp = ctx.enter_context(tc.tile_pool(name="hp", bufs=1))     # gathered h
    h2p = ctx.enter_context(tc.tile_pool(name="h2p", bufs=1))   # local h2
    scr = ctx.enter_context(tc.tile_pool(name="scr", bufs=2))   # LN scratch
    hloc = ctx.enter_context(tc.tile_pool(name="hloc", bufs=3))  # local h out
    qkp = ctx.enter_context(tc.tile_pool(name="qkp", bufs=1))
    expp = ctx.enter_context(tc.tile_pool(name="expp", bufs=6))
    otp = ctx.enter_context(tc.tile_pool(name="otp", bufs=1))
    up = ctx.enter_context(tc.tile_pool(name="up", bufs=1))     # mlp hidden
    zcp = ctx.enter_context(tc.tile_pool(name="zcp", bufs=3))   # d1 bf16 cast
    zfp = ctx.enter_context(tc.tile_pool(name="zfp", bufs=3))   # d1 readback
    lgp = ctx.enter_context(tc.tile_pool(name="lgp", bufs=2))
    hwp = ctx.enter_context(tc.tile_pool(name="hwp", bufs=2))
    rows = ctx.enter_context(tc.tile_pool(name="rows", bufs=1))
    arow = ctx.enter_context(tc.tile_pool(name="arow", bufs=2))
    rbp = ctx.enter_context(tc.tile_pool(name="rbp", bufs=2))
    psmm = ctx.enter_context(tc.tile_pool(name="psmm", bufs=4, space="PSUM"))
    psaux = ctx.enter_context(tc.tile_pool(name="psaux", bufs=3, space="PSUM"))
    psst = ctx.enter_context(tc.tile_pool(name="psst", bufs=1, space="PSUM"))
    dram = ctx.enter_context(tc.tile_pool(name="dram", bufs=1, space="DRAM"))

    # ---------------- constants ----------------
    inv_col = sing.tile([128, 1], BF, name="inv_col")
    nc.vector.memset(inv_col, 1.0 / D)
    ones_row = sing.tile([1, 128], BF, name="ones_row")
    nc.vector.memset(ones_row, 1.0)
    eps_ap = sing.tile([1, 1], F32, name="eps_ap")
    nc.vector.memset(eps_ap, EPS)
    wfill = sing.tile([128, TCH], BF, name="wfill")
    nc.vector.memset(wfill, 1.0)

    def warm(n):
        """Emit n dummy matmuls that run on the PE during an upcoming stall,
        keeping the HAM clock gate at 2.4 GHz."""
        if n <= 0:
            return
        ps_w = psaux.tile([1, TCH], F32, name="warm", tag="aux")
        for _ in range(n):
            nc.tensor.matmul(ps_w, inv_col, wfill, start=True, stop=True)

    # ---------------- residual stream (own tokens, transposed) ----------
    x = [sing.tile([128, TS], F32, name=f"x{k}") for k in range(KT)]
    for k in range(KT):
        nc.sync.dma_start(out=x[k], in_=xT0[k * 128:(k + 1) * 128, :])

    # ---------------- layernorm on own tokens ----------------
    def layernorm(grow_dram, brow_dram, name, sink, make_tile=None):
        """LN over the feature (partition) axis of transposed activations.
        Stats via one fused [x|x^2] matmul per k-tile. For each k, writes the
        bf16 normalized tile [128, TS] into make_tile(k) (default: hloc
        rotation) and calls sink(k, tile)."""
        if make_tile is None:
            make_tile = lambda k: hloc.tile([128, TS], BF, name="hk",
                                            tag="hk")
        gcol = tiny.tile([128, KT], F32, name=f"g_{name}", tag="gcol")
        bcol = tiny.tile([128, KT], F32, name=f"b_{name}", tag="bcol")
        nc.sync.dma_start(out=gcol, in_=grow_dram)
        nc.sync.dma_start(out=bcol, in_=brow_dram)

        ps_st = psst.tile([1, 2 * TS], F32, name="ps_st", tag="ps_st")
        for k in range(KT):
            xs = scr.tile([128, 2 * TS], BF, name="xs", tag="xs")
            nc.vector.tensor_copy(xs[:, 0:TS], x[k])
            nc.vector.tensor_tensor(out=xs[:, TS:2 * TS], in0=xs[:, 0:TS],
                                    in1=xs[:, 0:TS], op=AL.mult)
            nc.tensor.matmul(ps_st, inv_col, xs, start=(k == 0),
                             stop=(k == KT - 1))
        warm(10)
        # moments: mean | E[x^2] in one [1, 2*TS] row
        mom = rows.tile([1, 2 * TS], F32, name=f"mom_{name}", tag="mom")
        nc.vector.tensor_copy(mom, ps_st)
        mean = mom[:, 0:TS]
        msq = mom[:, TS:2 * TS]
        var = rows.tile([1, TS], F32, name=f"var_{name}", tag="var")
        # var = -(mean*mean) + msq
        nc.vector.scalar_tensor_tensor(out=var, in0=mean, scalar=-1.0,
                                       in1=mean, op0=AL.mult, op1=AL.mult)
        nc.vector.tensor_tensor(out=var, in0=var, in1=msq, op=AL.add)
        sd = rows.tile([1, TS], F32, name=f"sd_{name}", tag="sd")
        nc.scalar.activation(sd, var, AF.Sqrt, bias=eps_ap)
        rstd = rows.tile([1, TS], F32, name=f"rstd_{name}", tag="rstd")
        rsc = rows.tile([1, TS], F32, name=f"rsc_{name}", tag="rsc")
        nc.vector.reciprocal_approx_accurate(rstd, sd, rsc)
        nmrb = rows.tile([1, TS], BF, name=f"nmr_{name}", tag="nmr")
        nc.vector.scalar_tensor_tensor(out=nmrb, in0=mean, scalar=-1.0,
                                       in1=rstd, op0=AL.mult, op1=AL.mult)
        rstdb = rows.tile([1, TS], BF, name=f"rstdb_{name}", tag="rstdb")
        nc.vector.tensor_copy(rstdb, rstd)
        # broadcast to [128, TS] via K=1 outer-product matmuls (bf16)
        pbr = psaux.tile([128, TS], F32, name="pbr", tag="aux")
        nc.tensor.matmul(pbr, ones_row, rstdb, start=True, stop=True)
        pbn = psaux.tile([128, TS], F32, name="pbn", tag="aux")
        nc.tensor.matmul(pbn, ones_row, nmrb, start=True, stop=True)
        # apply: h = (x*rstdB + nmB)*g + b, reading broadcasts from PSUM
        for k in range(KT):
            t1 = scr.tile([128, TS], F32, name="lnt", tag="lnt")
            nc.vector.tensor_tensor(out=t1, in0=x[k], in1=pbr, op=AL.mult)
            t2 = scr.tile([128, TS], BF, name="lnt2", tag="lnt2")
            nc.vector.tensor_tensor(out=t2, in0=t1, in1=pbn, op=AL.add)
            ht = make_tile(k)
            nc.vector.tensor_scalar(
                out=ht, in0=t2, scalar1=gcol[:, k:k + 1],
                scalar2=bcol[:, k:k + 1], op0=AL.mult, op1=AL.add)
            sink(k, ht)

    def ln_gather(grow_dram, brow_dram, name, lsuf):
        """LN + one AllGather over the group; returns the DRAM out tensor."""
        h_in = dram.tile([D, TS], BF, name=f"h{lsuf}_in")
        h_out = dram.tile([TP, D, TS], BF, name=f"h{lsuf}_out")

        def sink(k, ht):
            nc.sync.dma_start(out=h_in[k * 128:(k + 1) * 128, :], in_=ht)
            if k == KT - 1:
                nc.gpsimd.collective_compute(
                    "AllGather", AL.bypass, replica_groups=groups,
                    ins=[h_in.opt()], outs=[h_out.opt()])

        layernorm(grow_dram, brow_dram, name, sink)
        return h_out

    def read_gathered(h_out, k, pool_tag):
        """Read k-tile [128, T] (all tokens, global order) of a gathered h."""
        ht = hp.tile([128, TP, TS], BF, name=f"ht{k}", tag=f"{pool_tag}{k}")
        nc.sync.dma_start(
            out=ht,
            in_=h_out[:, k * 128:(k + 1) * 128, :].rearrange(
                "r p t -> p r t"))
        return ht.rearrange("p r t -> p (r t)")

    # ---------------- transformer layers ----------------
    for l in range(L):
        # -- attention weights for this layer (contiguous pre-tiled loads) --
        wqt = wts.tile([128, KT, DSH], BF, name="wqt", tag="wqt")
        wkt = wts.tile([128, KT, DSH], BF, name="wkt", tag="wkt")
        wvt = wts.tile([128, KT, DSH], BF, name="wvt", tag="wvt")
        for dst, src in ((wqt, wq), (wkt, wk), (wvt, wv)):
            nc.sync.dma_start(out=dst,
                              in_=src[l].rearrange("p (k m) -> p k m", k=KT))
        wot = wts.tile([128, MSH, D], BF, name="wot", tag="wot")
        nc.sync.dma_start(out=wot,
                          in_=wo[l].rearrange("p (k m) -> p k m", k=MSH))
        b1col = tiny.tile([128, KF], F32, name="b1col", tag="b1col")
        nc.sync.dma_start(out=b1col, in_=b1d[l])
        b2col = tiny.tile([128, KT], F32, name="b2col", tag="b2col")
        nc.sync.dma_start(out=b2col, in_=b2d[l])

        # -- LN1 + AG --
        h1_out = ln_gather(g1d[l], be1d[l], f"ln1_{l}", f"1_{l}")
        warm(cfgc["warm_ag"])
        h1 = [read_gathered(h1_out, k, "h") for k in range(KT)]
        if dbg and l == 0:
            for k in range(KT):
                nc.sync.dma_start(out=dbg["h1"][k * 128:(k + 1) * 128, :],
                                  in_=h1[k])

        # -- QKV projections --
        # qT/kT: [DSH, T] transposed; v: natural [T, DSH] + ones column
        qT = [qkp.tile([128, T], BF, name=f"qT{m}", tag=f"qT{m}")
              for m in range(MSH)]
        kTt = [qkp.tile([128, T], BF, name=f"kT{m}", tag=f"kT{m}")
               for m in range(MSH)]
        for wt, dst in ((wqt, qT), (wkt, kTt)):
            pq = {}
            for m in range(MSH):
                for chn in range(NCH):
                    pq[(m, chn)] = psmm.tile([128, TCH], F32, name="ps",
                                             tag="mm")
            for k in range(KT):
                for m in range(MSH):
                    for chn in range(NCH):
                        cs = slice(chn * TCH, (chn + 1) * TCH)
                        nc.tensor.matmul(pq[(m, chn)],
                                         wt[:, k, m * 128:(m + 1) * 128],
                                         h1[k][:, cs],
                                         start=(k == 0), stop=(k == KT - 1))
            for m in range(MSH):
                for chn in range(NCH):
                    cs = slice(chn * TCH, (chn + 1) * TCH)
                    nc.vector.tensor_copy(dst[m][:, cs], pq[(m, chn)])
        vt = qkp.tile([128, TT, HL, DK + 1], BF, name="vt", tag="vt")
        nc.vector.memset(vt[:, :, :, DK:DK + 1], 1.0)
        for t in range(TT):
            ps = psmm.tile([128, TCH], F32, name="psv", tag="mm")
            for k in range(KT):
                nc.tensor.matmul(ps[:, 0:DSH],
                                 h1[k][:, t * 128:(t + 1) * 128],
                                 wvt[:, k, :],
                                 start=(k == 0), stop=(k == KT - 1))
            nc.vector.tensor_copy(
                vt[:, t, :, 0:DK],
                ps[:, 0:DSH].rearrange("p (h d) -> p h d", h=HL))

        # -- attention, software-pipelined division (depth 2) --
        oT = [otp.tile([128, T], BF, name=f"oT{m}", tag=f"oT{m}")
              for m in range(MSH)]
        units = [(hh, chn) for hh in range(HL) for chn in range(NCH)]
        pend = []

        def divide(hh, chn, ps_o):
            mt = (hh * DK) // 128
            po = (hh * DK) % 128
            cs = slice(chn * TCH, (chn + 1) * TCH)
            den = arow.tile([1, TCH], F32, name="den", tag="den")
            nc.vector.tensor_copy(den, ps_o[DK:DK + 1, :])
            rec = arow.tile([1, TCH], F32, name="rec", tag="rec")
            rsc2 = arow.tile([1, TCH], F32, name="rsc2", tag="rsc2")
            nc.vector.reciprocal_approx_accurate(rec, den, rsc2)
            recb = arow.tile([1, TCH], BF, name="recb", tag="recb")
            nc.vector.tensor_copy(recb, rec)
            ps_r = psmm.tile([128, TCH], F32, name="ps_r", tag="mm")
            nc.tensor.matmul(ps_r[0:DK, :], ones_row[:, 0:DK], recb,
                             start=True, stop=True)
            rb = rbp.tile([DK, TCH], BF, name="rb", tag="rb")
            nc.scalar.copy(rb, ps_r[0:DK, :])
            nc.vector.tensor_tensor(
                out=oT[mt][po:po + DK, cs], in0=ps_o[0:DK, :], in1=rb,
                op=AL.mult)

        for hh, chn in units:
            mt = (hh * DK) // 128
            po = (hh * DK) % 128
            q_h = qT[mt][po:po + DK, :]
            k_h = kTt[mt][po:po + DK, :]
            cs = slice(chn * TCH, (chn + 1) * TCH)
            jmax = (chn + 1) * (TCH // 128)
            exps = []
            for j in range(jmax):
                pss = psmm.tile([128, TCH], F32, name="pss", tag="mm")
                nc.tensor.matmul(pss, k_h[:, j * 128:(j + 1) * 128],
                                 q_h[:, cs], start=True, stop=True)
                et = expp.tile([128, TCH], BF, name="exp", tag="exp")
                nc.scalar.activation(et, pss, AF.Exp, scale=0.125)
                if j * 128 >= chn * TCH:
                    # diagonal block: zero where tk_global > tq_global
                    nc.gpsimd.affine_select(
                        out=et, in_=et, pattern=[[1, TCH]],
                        compare_op=AL.is_ge, fill=0.0,
                        base=chn * TCH - j * 128, channel_multiplier=-1)
                exps.append(et)
            ps_o = psaux.tile([DK + 1, TCH], F32, name="ps_o", tag="aux")
            for j in range(jmax):
                nc.tensor.matmul(ps_o, vt[:, j, hh, :], exps[j],
                                 start=(j == 0), stop=(j == jmax - 1))
            pend.append((hh, chn, ps_o))
            if len(pend) > 2:
                divide(*pend.pop(0))
        for p_ in pend:
            divide(*p_)
        if dbg and l == 0:
            for m in range(MSH):
                nc.sync.dma_start(out=dbg["o"][m * 128:(m + 1) * 128, :],
                                  in_=oT[m])

        # -- attn out: row-parallel Wo + single ReduceScatter (bf16) --
        d_in = dram.tile([TP, D, TS], BF, name=f"d_in{l}")
        d_out = dram.tile([D, TS], BF, name=f"d_out{l}")
        for mo in range(KT):
            for chn in range(NCH):
                cs = slice(chn * TCH, (chn + 1) * TCH)
                psd = psmm.tile([128, TCH], F32, name="psd", tag="mm")
                for k in range(MSH):
                    nc.tensor.matmul(psd,
                                     wot[:, k, mo * 128:(mo + 1) * 128],
                                     oT[k][:, cs],
                                     start=(k == 0), stop=(k == MSH - 1))
                dc = zcp.tile([128, TCH], BF, name="dc", tag="zc")
                nc.vector.tensor_copy(dc, psd)
                for tb in range(TCH // TS):
                    gtb = chn * (TCH // TS) + tb
                    nc.sync.dma_start(
                        out=d_in[gtb, mo * 128:(mo + 1) * 128, :],
                        in_=dc[:, tb * TS:(tb + 1) * TS])
        nc.gpsimd.collective_compute(
            "ReduceScatter", AL.add, replica_groups=groups,
            ins=[d_in.opt()], outs=[d_out.opt()])
        warm(cfgc["warm_rs"])
        for k in range(KT):
            df = zfp.tile([128, TS], BF, name="df", tag="zf")
            nc.sync.dma_start(out=df,
                              in_=d_out[k * 128:(k + 1) * 128, :])
            nc.vector.tensor_tensor(out=x[k], in0=x[k], in1=df, op=AL.add)
        if dbg and l == 0:
            for k in range(KT):
                nc.sync.dma_start(out=dbg["x1"][k * 128:(k + 1) * 128, :],
                                  in_=x[k])

        # -- LN2 (local) + fully local MLP on own tokens, full weights --
        h2 = [None] * KT

        def sink_h2(k, ht):
            h2[k] = ht

        layernorm(g2d[l], be2d[l], f"ln2_{l}", sink_h2,
                  make_tile=lambda k: h2p.tile([128, TS], BF, name=f"h2_{k}",
                                               tag=f"h2_{k}"))

        # MLP up: u[m] = gelu(W1[:, m]^T h2 + b1[m]), m over 32 hidden tiles
        ut = []
        for mc in range(KF // MW):
            w1c = w1p.tile([128, KT, MW * 128], BF, name="w1c", tag="w1c")
            nc.sync.dma_start(
                out=w1c,
                in_=w1[l].rearrange("p (k m) -> p k m", k=KT)
                [:, :, mc * MW * 128:(mc + 1) * MW * 128])
            for mi in range(MW):
                m = mc * MW + mi
                psu = psmm.tile([128, TCH], F32, name="psu", tag="mm")
                for k in range(KT):
                    nc.tensor.matmul(psu[:, 0:TS],
                                     w1c[:, k, mi * 128:(mi + 1) * 128],
                                     h2[k],
                                     start=(k == 0), stop=(k == KT - 1))
                u = up.tile([128, TS], BF, name=f"u{m}", tag=f"u{m}")
                if cfgc.get("gelu_sim"):
                    u0 = scr.tile([128, TS], F32, name="u0", tag="u0")
                    nc.vector.tensor_scalar_add(u0, psu[:, 0:TS],
                                                b1col[:, m:m + 1])
                    sg = scr.tile([128, TS], F32, name="sg", tag="sg")
                    nc.scalar.activation(sg, u0, AF.Sigmoid, scale=1.702)
                    nc.vector.tensor_mul(u, u0, sg)
                else:
                    nc.scalar.activation(u, psu[:, 0:TS], AF.Gelu,
                                         bias=b1col[:, m:m + 1])
                ut.append(u)

        # MLP down: x[m] += W2[:, m]^T u + b2[m], m over 8 feature tiles
        for m in range(KT):
            w2m = w2p.tile([128, KF, 128], BF, name="w2m", tag="w2m")
            nc.sync.dma_start(
                out=w2m,
                in_=w2[l][:, m, :].rearrange("p (k j) -> p k j", k=KF))
            psz = psmm.tile([128, TCH], F32, name="psz", tag="mm")
            for k in range(KF):
                nc.tensor.matmul(psz[:, 0:TS], w2m[:, k, :], ut[k],
                                 start=(k == 0), stop=(k == KF - 1))
            # x = (z + b2) + x, straight from PSUM
            nc.vector.scalar_tensor_tensor(
                out=x[m], in0=psz[:, 0:TS], scalar=b2col[:, m:m + 1],
                in1=x[m], op0=AL.add, op1=AL.add)
        if dbg and l == 0:
            for k in range(KT):
                nc.sync.dma_start(out=dbg["x2"][k * 128:(k + 1) * 128, :],
                                  in_=x[k])

    # ---------------- final LN + logits ----------------
    hf_out = ln_gather(gfd[:, :], befd[:, :], "lnf", "f")
    warm(cfgc["warm_f"])
    hf = [read_gathered(hf_out, k, "h") for k in range(KT)]
    for n in range(NV):
        hwn = hwp.tile([128, KT, VCH], BF, name="hwn", tag="hwn")
        nc.sync.dma_start(out=hwn, in_=hwd[:, n])
        for t in range(TT):
            ps = psmm.tile([128, TCH], F32, name="pslg", tag="mm")
            for k in range(KT):
                nc.tensor.matmul(ps[:, 0:VCH],
                                 hf[k][:, t * 128:(t + 1) * 128],
                                 hwn[:, k, :],
                                 start=(k == 0), stop=(k == KT - 1))
            lg = lgp.tile([128, VCH], F32, name="lg", tag="lg")
            nc.vector.tensor_copy(lg, ps[:, 0:VCH])
            nc.sync.dma_start(
                out=logits[t * 128:(t + 1) * 128, n * VCH:(n + 1) * VCH],
                in_=lg)

    ctx.close()


# ---------------- host side ----------------

_PROG_CACHE = {}


def _get_program():
    if "nc" not in _PROG_CACHE:
        _PROG_CACHE["nc"] = build_program()
    return _PROG_CACHE["nc"]


def _tile_kp(w):
    """[K*128, M] -> [128, K*M] pre-tiled layout: out[p, k*M+m] = w[k*128+p, m]."""
    kt = w.shape[0] // 128
    return np.ascontiguousarray(
        w.reshape(kt, 128, -1).transpose(1, 0, 2).reshape(128, -1))


def _col(vv):
    """[K*128] -> [128, K] column layout."""
    return np.ascontiguousarray(vv.reshape(-1, 128).T)


def make_in_maps(input_ids, emb, Wq, Wk, Wv, Wo, W1, b1, W2, b2,
                 ln1_g, ln1_b, ln2_g, ln2_b, lnf_g, lnf_b, head_w, cfg=None):
    c_ = dict(CFG)
    if cfg:
        c_.update(cfg)
    TP = c_["TP"]
    D, V, L, S = c_["D"], c_["V"], c_["L"], c_["S"]
    TS = S // TP
    DF = 4 * D
    DSH, VSH = D // TP, V // TP
    KT = D // 128
    KF = DF // 128
    VCH = 500
    NV = VSH // VCH
    bf = ml_dtypes.bfloat16
    f32 = np.float32
    emb = np.asarray(emb)
    input_ids = np.asarray(input_ids)

    def w2_layout(w):
        # [DF, D] -> [128, KT, KF*128]: out[p, m, k*128+j] = w[k*128+p, m*128+j]
        return np.ascontiguousarray(
            w.reshape(KF, 128, KT, 128).transpose(1, 2, 0, 3)
            .reshape(128, KT, KF * 128))

    in_maps = []
    for c in range(N_CORES):
        g, r = c // TP, c % TP
        x0 = emb[input_ids[g]]                                   # [S, D] f32
        xT = np.ascontiguousarray(x0.T[:, r * TS:(r + 1) * TS]).astype(f32)
        m = {
            "xT0": xT,
            "wq": np.stack([_tile_kp(np.asarray(Wq[l])[:, r * DSH:(r + 1) * DSH])
                            for l in range(L)]).astype(bf),
            "wk": np.stack([_tile_kp(np.asarray(Wk[l])[:, r * DSH:(r + 1) * DSH])
                            for l in range(L)]).astype(bf),
            "wv": np.stack([_tile_kp(np.asarray(Wv[l])[:, r * DSH:(r + 1) * DSH])
                            for l in range(L)]).astype(bf),
            "wo": np.stack([_tile_kp(np.asarray(Wo[l])[r * DSH:(r + 1) * DSH, :])
                            for l in range(L)]).astype(bf),
            "w1": np.stack([_tile_kp(np.asarray(W1[l]))
                            for l in range(L)]).astype(bf),
            "w2": np.stack([w2_layout(np.asarray(W2[l]))
                            for l in range(L)]).astype(bf),
            "b1": np.stack([_col(np.asarray(b1[l])) for l in range(L)]).astype(f32),
            "b2": np.stack([_col(np.asarray(b2[l])) for l in range(L)]).astype(f32),
            "g1": np.stack([_col(np.asarray(ln1_g[l])) for l in range(L)]).astype(f32),
            "be1": np.stack([_col(np.asarray(ln1_b[l])) for l in range(L)]).astype(f32),
            "g2": np.stack([_col(np.asarray(ln2_g[l])) for l in range(L)]).astype(f32),
            "be2": np.stack([_col(np.asarray(ln2_b[l])) for l in range(L)]).astype(f32),
            "gf": _col(np.asarray(lnf_g)).astype(f32),
            "bef": _col(np.asarray(lnf_b)).astype(f32),
            "hw": np.ascontiguousarray(
                np.asarray(head_w)[:, r * VSH:(r + 1) * VSH]
                .reshape(KT, 128, NV, VCH).transpose(1, 2, 0, 3)).astype(bf),
        }
        in_maps.append(m)
    return in_maps


def _assemble(res):
    B, S, V = CFG["B"], CFG["S"], CFG["V"]
    TP = CFG["TP"]
    VSH = V // TP
    out = np.empty((B, S, V), dtype=np.float32)
    for c in range(N_CORES):
        g, r = c // TP, c % TP
        out[g, :, r * VSH:(r + 1) * VSH] = res.results[c]["logits"]
    return out


def kernel(**inputs):
    nc = _get_program()
    in_maps = make_in_maps(**inputs)
    res = run_bass_kernel_spmd(nc, in_maps, list(range(N_CORES)), trace=False)
    return _assemble(res)


def run_traced(**inputs):
    """Like kernel() but with NTFF tracing; returns (out, exec_time_ns)."""
    nc = _get_program()
    in_maps = make_in_maps(**inputs)
    res = run_bass_kernel_spmd(nc, in_maps, list(range(N_CORES)), trace=True)
    return _assemble(res), res.exec_time_ns


# revision 35
# speedup vs baseline: 1.0029x; 1.0029x over previous
"""Bass/Tile kernel for a 4-layer dense transformer (prefill) on 8 TRN2 cores.

Parallelization: 2-way data parallel (batch) x 4-way tensor parallel with a
sequence-parallel residual stream (Megatron-SP style).
Groups: cores [0,1,2,3] handle batch 0, [4,5,6,7] batch 1.
Within a group (rank r = core % 4):
  - residual stream x: OWN tokens r*256..(r+1)*256, transposed [D, 256] f32
  - layernorms computed on the 256 own tokens only
  - attention: heads r*4..r*4+3 over all 1024 tokens (h1 AllGathered)
  - attn out projection: row-parallel Wo partial over all tokens;
    ReduceScatter (bf16) delivers each rank the full-D delta for its tokens
  - MLP: fully LOCAL on own tokens with full W1/W2 streamed from HBM
    (no collective at all; weights are replicated instead of sharded)
  - vocab: cols r*8000..(r+1)*8000 of head_w for the logits
Collectives per layer: AG(h1 bf16 0.5MB) + RS(d1 bf16 2MB) only.
PE keep-warm dummy matmul chains run during the collective stalls so the
HAM clock gate stays at 2.4 GHz.
Final logits are computed in natural [token, vocab] layout and written out
per-core as [1024, 8000]; the host concatenates.
"""

import sys
import types

import numpy as np


def _install_ntff_shim():
    """Register the NTFF profiling hook that trn_boot skipped (the image's
    antenv package lacks the axon_hooks submodule)."""
    if "antenv.axon_hooks" in sys.modules:
        return
    try:
        import trn_agent_boot.trn_boot as tb
        hook = tb._ntff_profile_via_ctypes("/opt/axon/libaxon_pjrt.so")
    except Exception:
        hook = None
    mod = types.ModuleType("antenv.axon_hooks")
    _h = [hook]
    mod.get_axon_ntff_profile_hook = lambda: _h[0]
    mod.set_axon_ntff_profile_hook = lambda h: _h.__setitem__(0, h)
    sys.modules["antenv.axon_hooks"] = mod
    try:
        import antenv
        antenv.axon_hooks = mod
    except Exception:
        pass


_install_ntff_shim()

import ml_dtypes
import concourse.bass as bass
import concourse.mybir as mybir
import concourse.tile as tile
from concourse import bacc
from concourse.bass_utils import run_bass_kernel_spmd

BF = mybir.dt.bfloat16
F32 = mybir.dt.float32
AL = mybir.AluOpType
AF = mybir.ActivationFunctionType

# Model sizes (full problem, hardcoded per contract).
CFG = dict(
    B=2, S=1024, V=32000, D=1024, H=16, L=4, EPS=1e-5,
    TP=4,            # tensor-parallel width (group size)
    gelu_sim=False,  # CoreSim lacks Gelu; use sigmoid-based stand-in
    warm_ag=36,      # keep-warm matmuls during AG(h1)
    warm_rs=70,      # keep-warm matmuls during RS(d1)
    warm_f=60,       # keep-warm matmuls during final AG
)

N_CORES = 8
GROUPS = [[0, 1, 2, 3], [4, 5, 6, 7]]


def build_program(cfg=None):
    """Build the SPMD Bass program (identical on all 8 cores)."""
    c = dict(CFG)
    if cfg:
        c.update(cfg)
    B, S, V, D, H, L = c["B"], c["S"], c["V"], c["D"], c["H"], c["L"]
    EPS, TP = c["EPS"], c["TP"]
    T = S                    # tokens per group (one batch element)
    TS = T // TP             # own tokens per rank (256)
    DK = D // H              # head dim (64)
    HL = H // TP             # heads per core (4)
    DSH = D // TP            # attention feature shard (256)
    DF = 4 * D
    VSH = V // TP            # vocab shard (8000)
    KT = D // 128            # feature k-tiles (8)
    KF = DF // 128           # full mlp hidden tiles (32)
    NCH = max(1, T // 512)   # token chunks of <=512
    TCH = min(512, T)        # token chunk size
    MSH = DSH // 128         # m-tiles of a DSH-wide slab (2)
    TT = T // 128            # token tiles (8)
    VCH = 500                # vocab chunk
    NV = VSH // VCH          # vocab n-chunks (16)
    MW = 4                   # w1 m-tiles per streamed chunk
    assert T % 128 == 0 and D % 128 == 0 and DSH % 128 == 0 and TS % 128 == 0

    groups = [[g * TP + r for r in range(TP)] for g in range(N_CORES // TP)]

    nc = bacc.Bacc("TRN2", target_bir_lowering=False, debug=False,
                   num_devices=N_CORES)

    # ---- DRAM parameters (per-core shards fed via in_maps, pre-tiled) ----
    xT0 = nc.dram_tensor("xT0", [D, TS], F32, kind="ExternalInput")
    wq = nc.dram_tensor("wq", [L, 128, KT * DSH], BF, kind="ExternalInput")
    wk = nc.dram_tensor("wk", [L, 128, KT * DSH], BF, kind="ExternalInput")
    wv = nc.dram_tensor("wv", [L, 128, KT * DSH], BF, kind="ExternalInput")
    wo = nc.dram_tensor("wo", [L, 128, MSH * D], BF, kind="ExternalInput")
    w1 = nc.dram_tensor("w1", [L, 128, KT * DF], BF, kind="ExternalInput")
    w2 = nc.dram_tensor("w2", [L, 128, KT, KF * 128], BF,
                        kind="ExternalInput")
    b1 = nc.dram_tensor("b1", [L, 128, KF], F32, kind="ExternalInput")
    b2 = nc.dram_tensor("b2", [L, 128, KT], F32, kind="ExternalInput")
    g1 = nc.dram_tensor("g1", [L, 128, KT], F32, kind="ExternalInput")
    be1 = nc.dram_tensor("be1", [L, 128, KT], F32, kind="ExternalInput")
    g2 = nc.dram_tensor("g2", [L, 128, KT], F32, kind="ExternalInput")
    be2 = nc.dram_tensor("be2", [L, 128, KT], F32, kind="ExternalInput")
    gf = nc.dram_tensor("gf", [128, KT], F32, kind="ExternalInput")
    bef = nc.dram_tensor("bef", [128, KT], F32, kind="ExternalInput")
    hw = nc.dram_tensor("hw", [128, NV, KT, VCH], BF, kind="ExternalInput")
    logits = nc.dram_tensor("logits", [T, VSH], F32, kind="ExternalOutput")
    dbg = {}
    if c.get("dbg"):
        dbg["h1"] = nc.dram_tensor("dbg_h1", [D, T], BF, kind="ExternalOutput")
        dbg["o"] = nc.dram_tensor("dbg_o", [DSH, T], BF, kind="ExternalOutput")
        dbg["x1"] = nc.dram_tensor("dbg_x1", [D, TS], F32,
                                   kind="ExternalOutput")
        dbg["x2"] = nc.dram_tensor("dbg_x2", [D, TS], F32,
                                   kind="ExternalOutput")

    with tile.TileContext(nc) as tc:
        _build_tc(nc, tc, locals())
    nc.compile()
    return nc


def _build_tc(nc, tc, v):
    """Emit the tile program. `v` is the name->value dict from build_program."""
    (B, T, TS, D, L, EPS, TP, DK, HL, DSH, DF, VSH, KT, KF, NCH, TCH,
     MSH, NV, VCH, TT, MW, groups) = (
        v["B"], v["T"], v["TS"], v["D"], v["L"], v["EPS"], v["TP"], v["DK"],
        v["HL"], v["DSH"], v["DF"], v["VSH"], v["KT"], v["KF"], v["NCH"],
        v["TCH"], v["MSH"], v["NV"], v["VCH"], v["TT"], v["MW"], v["groups"])
    xT0, wq, wk, wv, wo, w1, w2 = (v["xT0"], v["wq"], v["wk"], v["wv"],
                                   v["wo"], v["w1"], v["w2"])
    b1d, b2d, g1d, be1d, g2d, be2d, gfd, befd = (
        v["b1"], v["b2"], v["g1"], v["be1"], v["g2"], v["be2"], v["gf"],
        v["bef"])
    hwd, logits, dbg = v["hw"], v["logits"], v["dbg"]
    cfgc = v["c"]

    import contextlib
    ctx = contextlib.ExitStack()

    # ---------------- pools ----------------
    sing = ctx.enter_context(tc.tile_pool(name="sing", bufs=1))
    wts = ctx.enter_context(tc.tile_pool(name="wts", bufs=1))
    w1p = ctx.enter_context(tc.tile_pool(name="w1p", bufs=3))   # w1 chunks
    w2p = ctx.enter_context(tc.tile_pool(name="w2p", bufs=3))   # w2 m-tiles
    tiny = ctx.enter_context(tc.tile_pool(name="tiny", bufs=2))
    hp = ctx.enter_context(tc.tile_pool(name="hp", bufs=1))     # gathered h
    h2p = ctx.enter_context(tc.tile_pool(name="h2p", bufs=1))   # local h2
    scr = ctx.enter_context(tc.tile_pool(name="scr", bufs=2))   # LN scratch
    hloc = ctx.enter_context(tc.tile_pool(name="hloc", bufs=3))  # local h out
    qkp = ctx.enter_context(tc.tile_pool(name="qkp", bufs=1))
    expp = ctx.enter_context(tc.tile_pool(name="expp", bufs=6))
    otp = ctx.enter_context(tc.tile_pool(name="otp", bufs=1))
    up = ctx.enter_context(tc.tile_pool(name="up", bufs=1))     # mlp hidden
    zcp = ctx.enter_context(tc.tile_pool(name="zcp", bufs=3))   # d1 bf16 cast
    zfp = ctx.enter_context(tc.tile_pool(name="zfp", bufs=3))   # d1 readback
    lgp = ctx.enter_context(tc.tile_pool(name="lgp", bufs=2))
    hwp = ctx.enter_context(tc.tile_pool(name="hwp", bufs=2))
    rows = ctx.enter_context(tc.tile_pool(name="rows", bufs=1))
    arow = ctx.enter_context(tc.tile_pool(name="arow", bufs=2))
    rbp = ctx.enter_context(tc.tile_pool(name="rbp", bufs=2))
    psmm = ctx.enter_context(tc.tile_pool(name="psmm", bufs=4, space="PSUM"))
    psaux = ctx.enter_context(tc.tile_pool(name="psaux", bufs=3, space="PSUM"))
    psst = ctx.enter_context(tc.tile_pool(name="psst", bufs=1, space="PSUM"))
    dram = ctx.enter_context(tc.tile_pool(name="dram", bufs=1, space="DRAM"))

    # ---------------- constants ----------------
    inv_col = sing.tile([128, 1], BF, name="inv_col")
    nc.vector.memset(inv_col, 1.0 / D)
    ones_row = sing.tile([1, 128], BF, name="ones_row")
    nc.vector.memset(ones_row, 1.0)
    eps_ap = sing.tile([1, 1], F32, name="eps_ap")
    nc.vector.memset(eps_ap, EPS)
    wfill = sing.tile([128, TCH], BF, name="wfill")
    nc.vector.memset(wfill, 1.0)

    def warm(n):
        """Emit n dummy matmuls that run on the PE during an upcoming stall,
        keeping the HAM clock gate at 2.4 GHz."""
        if n <= 0:
            return
        ps_w = psaux.tile([1, TCH], F32, name="warm", tag="aux")
        for _ in range(n):
            nc.tensor.matmul(ps_w, inv_col, wfill, start=True, stop=True)

    # ---------------- residual stream (own tokens, transposed) ----------
    x = [sing.tile([128, TS], F32, name=f"x{k}") for k in range(KT)]
    for k in range(KT):
        nc.sync.dma_start(out=x[k], in_=xT0[k * 128:(k + 1) * 128, :])

    # ---------------- layernorm on own tokens ----------------
    def layernorm(grow_dram, brow_dram, name, sink, make_tile=None):
        """LN over the feature (partition) axis of transposed activations.
        Stats via one fused [x|x^2] matmul per k-tile. For each k, writes the
        bf16 normalized tile [128, TS] into make_tile(k) (default: hloc
        rotation) and calls sink(k, tile)."""
        if make_tile is None:
            make_tile = lambda k: hloc.tile([128, TS], BF, name="hk",
                                            tag="hk")
        gcol = tiny.tile([128, KT], F32, name=f"g_{name}", tag="gcol")
        bcol = tiny.tile([128, KT], F32, name=f"b_{name}", tag="bcol")
        nc.sync.dma_start(out=gcol, in_=grow_dram)
        nc.sync.dma_start(out=bcol, in_=brow_dram)

        ps_st = psst.tile([1, 2 * TS], F32, name="ps_st", tag="ps_st")
        for k in range(KT):
            xs = scr.tile([128, 2 * TS], BF, name="xs", tag="xs")
            nc.vector.tensor_copy(xs[:, 0:TS], x[k])
            nc.vector.tensor_tensor(out=xs[:, TS:2 * TS], in0=xs[:, 0:TS],
                                    in1=xs[:, 0:TS], op=AL.mult)
            nc.tensor.matmul(ps_st, inv_col, xs, start=(k == 0),
                             stop=(k == KT - 1))
        # moments: mean | E[x^2] in one [1, 2*TS] row
        mom = rows.tile([1, 2 * TS], F32, name=f"mom_{name}", tag="mom")
        nc.vector.tensor_copy(mom, ps_st)
        mean = mom[:, 0:TS]
        msq = mom[:, TS:2 * TS]
        var = rows.tile([1, TS], F32, name=f"var_{name}", tag="var")
        # var = -(mean*mean) + msq
        nc.vector.scalar_tensor_tensor(out=var, in0=mean, scalar=-1.0,
                                       in1=mean, op0=AL.mult, op1=AL.mult)
        nc.vector.tensor_tensor(out=var, in0=var, in1=msq, op=AL.add)
        sd = rows.tile([1, TS], F32, name=f"sd_{name}", tag="sd")
        nc.scalar.activation(sd, var, AF.Sqrt, bias=eps_ap)
        rstd = rows.tile([1, TS], F32, name=f"rstd_{name}", tag="rstd")
        rsc = rows.tile([1, TS], F32, name=f"rsc_{name}", tag="rsc")
        nc.vector.reciprocal_approx_accurate(rstd, sd, rsc)
        nmrb = rows.tile([1, TS], BF, name=f"nmr_{name}", tag="nmr")
        nc.vector.scalar_tensor_tensor(out=nmrb, in0=mean, scalar=-1.0,
                                       in1=rstd, op0=AL.mult, op1=AL.mult)
        rstdb = rows.tile([1, TS], BF, name=f"rstdb_{name}", tag="rstdb")
        nc.vector.tensor_copy(rstdb, rstd)
        # broadcast to [128, TS] via K=1 outer-product matmuls (bf16)
        pbr = psaux.tile([128, TS], F32, name="pbr", tag="aux")
        nc.tensor.matmul(pbr, ones_row, rstdb, start=True, stop=True)
        pbn = psaux.tile([128, TS], F32, name="pbn", tag="aux")
        nc.tensor.matmul(pbn, ones_row, nmrb, start=True, stop=True)
        # apply: h = (x*rstdB + nmB)*g + b, reading broadcasts from PSUM
        for k in range(KT):
            t1 = scr.tile([128, TS], F32, name="lnt", tag="lnt")
            nc.vector.tensor_tensor(out=t1, in0=x[k], in1=pbr, op=AL.mult)
            t2 = scr.tile([128, TS], BF, name="lnt2", tag="lnt2")
            nc.vector.tensor_tensor(out=t2, in0=t1, in1=pbn, op=AL.add)
            ht = make_tile(k)
            nc.vector.tensor_scalar(
                out=ht, in0=t2, scalar1=gcol[:, k:k + 1],
                scalar2=bcol[:, k:k + 1], op0=AL.mult, op1=AL.add)
            sink(k, ht)

    def ln_gather(grow_dram, brow_dram, name, lsuf):
        """LN + one AllGather over the group; returns the DRAM out tensor."""
        h_in = dram.tile([D, TS], BF, name=f"h{lsuf}_in")
        h_out = dram.tile([TP, D, TS], BF, name=f"h{lsuf}_out")

        def sink(k, ht):
            nc.sync.dma_start(out=h_in[k * 128:(k + 1) * 128, :], in_=ht)
            if k == KT - 1:
                nc.gpsimd.collective_compute(
                    "AllGather", AL.bypass, replica_groups=groups,
                    ins=[h_in.opt()], outs=[h_out.opt()])

        layernorm(grow_dram, brow_dram, name, sink)
        return h_out

    def read_gathered(h_out, k, pool_tag):
        """Read k-tile [128, T] (all tokens, global order) of a gathered h."""
        ht = hp.tile([128, TP, TS], BF, name=f"ht{k}", tag=f"{pool_tag}{k}")
        nc.sync.dma_start(
            out=ht,
            in_=h_out[:, k * 128:(k + 1) * 128, :].rearrange(
                "r p t -> p r t"))
        return ht.rearrange("p r t -> p (r t)")

    # ---------------- transformer layers ----------------
    for l in range(L):
        # -- attention weights for this layer (contiguous pre-tiled loads) --
        wqt = wts.tile([128, KT, DSH], BF, name="wqt", tag="wqt")
        wkt = wts.tile([128, KT, DSH], BF, name="wkt", tag="wkt")
        wvt = wts.tile([128, KT, DSH], BF, name="wvt", tag="wvt")
        for dst, src in ((wqt, wq), (wkt, wk), (wvt, wv)):
            nc.sync.dma_start(out=dst,
                              in_=src[l].rearrange("p (k m) -> p k m", k=KT))
        wot = wts.tile([128, MSH, D], BF, name="wot", tag="wot")
        nc.sync.dma_start(out=wot,
                          in_=wo[l].rearrange("p (k m) -> p k m", k=MSH))
        b1col = tiny.tile([128, KF], F32, name="b1col", tag="b1col")
        nc.sync.dma_start(out=b1col, in_=b1d[l])
        b2col = tiny.tile([128, KT], F32, name="b2col", tag="b2col")
        nc.sync.dma_start(out=b2col, in_=b2d[l])

        # -- LN1 + AG --
        h1_out = ln_gather(g1d[l], be1d[l], f"ln1_{l}", f"1_{l}")
        warm(cfgc["warm_ag"])
        h1 = [read_gathered(h1_out, k, "h") for k in range(KT)]
        if dbg and l == 0:
            for k in range(KT):
                nc.sync.dma_start(out=dbg["h1"][k * 128:(k + 1) * 128, :],
                                  in_=h1[k])

        # -- QKV projections --
        # qT/kT: [DSH, T] transposed; v: natural [T, DSH] + ones column
        qT = [qkp.tile([128, T], BF, name=f"qT{m}", tag=f"qT{m}")
              for m in range(MSH)]
        kTt = [qkp.tile([128, T], BF, name=f"kT{m}", tag=f"kT{m}")
               for m in range(MSH)]
        for wt, dst in ((wqt, qT), (wkt, kTt)):
            pq = {}
            for m in range(MSH):
                for chn in range(NCH):
                    pq[(m, chn)] = psmm.tile([128, TCH], F32, name="ps",
                                             tag="mm")
            for k in range(KT):
                for m in range(MSH):
                    for chn in range(NCH):
                        cs = slice(chn * TCH, (chn + 1) * TCH)
                        nc.tensor.matmul(pq[(m, chn)],
                                         wt[:, k, m * 128:(m + 1) * 128],
                                         h1[k][:, cs],
                                         start=(k == 0), stop=(k == KT - 1))
            for m in range(MSH):
                for chn in range(NCH):
                    cs = slice(chn * TCH, (chn + 1) * TCH)
                    nc.vector.tensor_copy(dst[m][:, cs], pq[(m, chn)])
        vt = qkp.tile([128, TT, HL, DK + 1], BF, name="vt", tag="vt")
        nc.vector.memset(vt[:, :, :, DK:DK + 1], 1.0)
        for t in range(TT):
            ps = psmm.tile([128, TCH], F32, name="psv", tag="mm")
            for k in range(KT):
                nc.tensor.matmul(ps[:, 0:DSH],
                                 h1[k][:, t * 128:(t + 1) * 128],
                                 wvt[:, k, :],
                                 start=(k == 0), stop=(k == KT - 1))
            nc.vector.tensor_copy(
                vt[:, t, :, 0:DK],
                ps[:, 0:DSH].rearrange("p (h d) -> p h d", h=HL))

        # -- attention, software-pipelined division (depth 2) --
        oT = [otp.tile([128, T], BF, name=f"oT{m}", tag=f"oT{m}")
              for m in range(MSH)]
        units = [(hh, chn) for hh in range(HL) for chn in range(NCH)]
        pend = []

        def divide(hh, chn, ps_o):
            mt = (hh * DK) // 128
            po = (hh * DK) % 128
            cs = slice(chn * TCH, (chn + 1) * TCH)
            den = arow.tile([1, TCH], F32, name="den", tag="den")
            nc.vector.tensor_copy(den, ps_o[DK:DK + 1, :])
            rec = arow.tile([1, TCH], F32, name="rec", tag="rec")
            rsc2 = arow.tile([1, TCH], F32, name="rsc2", tag="rsc2")
            nc.vector.reciprocal_approx_accurate(rec, den, rsc2)
            recb = arow.tile([1, TCH], BF, name="recb", tag="recb")
            nc.vector.tensor_copy(recb, rec)
            ps_r = psmm.tile([128, TCH], F32, name="ps_r", tag="mm")
            nc.tensor.matmul(ps_r[0:DK, :], ones_row[:, 0:DK], recb,
                             start=True, stop=True)
            rb = rbp.tile([DK, TCH], BF, name="rb", tag="rb")
            nc.scalar.copy(rb, ps_r[0:DK, :])
            nc.vector.tensor_tensor(
                out=oT[mt][po:po + DK, cs], in0=ps_o[0:DK, :], in1=rb,
                op=AL.mult)

        for hh, chn in units:
            mt = (hh * DK) // 128
            po = (hh * DK) % 128
            q_h = qT[mt][po:po + DK, :]
            k_h = kTt[mt][po:po + DK, :]
            cs = slice(chn * TCH, (chn + 1) * TCH)
            jmax = (chn + 1) * (TCH // 128)
            exps = []
            for j in range(jmax):
                pss = psmm.tile([128, TCH], F32, name="pss", tag="mm")
                nc.tensor.matmul(pss, k_h[:, j * 128:(j + 1) * 128],
                                 q_h[:, cs], start=True, stop=True)
                et = expp.tile([128, TCH], BF, name="exp", tag="exp")
                nc.scalar.activation(et, pss, AF.Exp, scale=0.125)
                if j * 128 >= chn * TCH:
                    # diagonal block: zero where tk_global > tq_global
                    nc.gpsimd.affine_select(
                        out=et, in_=et, pattern=[[1, TCH]],
                        compare_op=AL.is_ge, fill=0.0,
                        base=chn * TCH - j * 128, channel_multiplier=-1)
                exps.append(et)
            ps_o = psaux.tile([DK + 1, TCH], F32, name="ps_o", tag="aux")
            for j in range(jmax):
                nc.tensor.matmul(ps_o, vt[:, j, hh, :], exps[j],
                                 start=(j == 0), stop=(j == jmax - 1))
            pend.append((hh, chn, ps_o))
            if len(pend) > 2:
                divide(*pend.pop(0))
        for p_ in pend:
            divide(*p_)
        if dbg and l == 0:
            for m in range(MSH):
                nc.sync.dma_start(out=dbg["o"][m * 128:(m + 1) * 128, :],
                                  in_=oT[m])

        # -- attn out: row-parallel Wo + single ReduceScatter (bf16) --
        d_in = dram.tile([TP, D, TS], BF, name=f"d_in{l}")
        d_out = dram.tile([D, TS], BF, name=f"d_out{l}")
        for mo in range(KT):
            for chn in range(NCH):
                cs = slice(chn * TCH, (chn + 1) * TCH)
                psd = psmm.tile([128, TCH], F32, name="psd", tag="mm")
                for k in range(MSH):
                    nc.tensor.matmul(psd,
                                     wot[:, k, mo * 128:(mo + 1) * 128],
                                     oT[k][:, cs],
                                     start=(k == 0), stop=(k == MSH - 1))
                dc = zcp.tile([128, TCH], BF, name="dc", tag="zc")
                nc.vector.tensor_copy(dc, psd)
                for tb in range(TCH // TS):
                    gtb = chn * (TCH // TS) + tb
                    nc.sync.dma_start(
                        out=d_in[gtb, mo * 128:(mo + 1) * 128, :],
                        in_=dc[:, tb * TS:(tb + 1) * TS])
        nc.gpsimd.collective_compute(
            "ReduceScatter", AL.add, replica_groups=groups,
            ins=[d_in.opt()], outs=[d_out.opt()])
        warm(cfgc["warm_rs"])
        for k in range(KT):
            df = zfp.tile([128, TS], BF, name="df", tag="zf")
            nc.sync.dma_start(out=df,
                              in_=d_out[k * 128:(k + 1) * 128, :])
            nc.vector.tensor_tensor(out=x[k], in0=x[k], in1=df, op=AL.add)
        if dbg and l == 0:
            for k in range(KT):
                nc.sync.dma_start(out=dbg["x1"][k * 128:(k + 1) * 128, :],
                                  in_=x[k])

        # -- LN2 (local) + fully local MLP on own tokens, full weights --
        h2 = [None] * KT

        def sink_h2(k, ht):
            h2[k] = ht

        layernorm(g2d[l], be2d[l], f"ln2_{l}", sink_h2,
                  make_tile=lambda k: h2p.tile([128, TS], BF, name=f"h2_{k}",
                                               tag=f"h2_{k}"))

        # MLP up: u[m] = gelu(W1[:, m]^T h2 + b1[m]), m over 32 hidden tiles
        ut = []
        for mc in range(KF // MW):
            w1c = w1p.tile([128, KT, MW * 128], BF, name="w1c", tag="w1c")
            nc.sync.dma_start(
                out=w1c,
                in_=w1[l].rearrange("p (k m) -> p k m", k=KT)
                [:, :, mc * MW * 128:(mc + 1) * MW * 128])
            for mi in range(MW):
                m = mc * MW + mi
                psu = psmm.tile([128, TCH], F32, name="psu", tag="mm")
                for k in range(KT):
                    nc.tensor.matmul(psu[:, 0:TS],
                                     w1c[:, k, mi * 128:(mi + 1) * 128],
                                     h2[k],
                                     start=(k == 0), stop=(k == KT - 1))
                u = up.tile([128, TS], BF, name=f"u{m}", tag=f"u{m}")
                if cfgc.get("gelu_sim"):
                    u0 = scr.tile([128, TS], F32, name="u0", tag="u0")
                    nc.vector.tensor_scalar_add(u0, psu[:, 0:TS],
                                                b1col[:, m:m + 1])
                    sg = scr.tile([128, TS], F32, name="sg", tag="sg")
                    nc.scalar.activation(sg, u0, AF.Sigmoid, scale=1.702)
                    nc.vector.tensor_mul(u, u0, sg)
                else:
                    nc.scalar.activation(u, psu[:, 0:TS], AF.Gelu,
                                         bias=b1col[:, m:m + 1])
                ut.append(u)

        # MLP down: x[m] += W2[:, m]^T u + b2[m], m over 8 feature tiles
        for m in range(KT):
            w2m = w2p.tile([128, KF, 128], BF, name="w2m", tag="w2m")
            nc.sync.dma_start(
                out=w2m,
                in_=w2[l][:, m, :].rearrange("p (k j) -> p k j", k=KF))
            psz = psmm.tile([128, TCH], F32, name="psz", tag="mm")
            for k in range(KF):
                nc.tensor.matmul(psz[:, 0:TS], w2m[:, k, :], ut[k],
                                 start=(k == 0), stop=(k == KF - 1))
            # x = (z + b2) + x, straight from PSUM
            nc.vector.scalar_tensor_tensor(
                out=x[m], in0=psz[:, 0:TS], scalar=b2col[:, m:m + 1],
                in1=x[m], op0=AL.add, op1=AL.add)
        if dbg and l == 0:
            for k in range(KT):
                nc.sync.dma_start(out=dbg["x2"][k * 128:(k + 1) * 128, :],
                                  in_=x[k])

    # ---------------- final LN + logits ----------------
    hf_out = ln_gather(gfd[:, :], befd[:, :], "lnf", "f")
    warm(cfgc["warm_f"])
    hf = [read_gathered(hf_out, k, "h") for k in range(KT)]
    for n in range(NV):
        hwn = hwp.tile([128, KT, VCH], BF, name="hwn", tag="hwn")
        nc.sync.dma_start(out=hwn, in_=hwd[:, n])
        for t in range(TT):
            ps = psmm.tile([128, TCH], F32, name="pslg", tag="mm")
            for k in range(KT):
                nc.tensor.matmul(ps[:, 0:VCH],
                                 hf[k][:, t * 128:(t + 1) * 128],
                                 hwn[:, k, :],
                                 start=(k == 0), stop=(k == KT - 1))
            lg = lgp.tile([128, VCH], F32, name="lg", tag="lg")
            nc.vector.tensor_copy(lg, ps[:, 0:VCH])
            nc.sync.dma_start(
                out=logits[t * 128:(t + 1) * 128, n * VCH:(n + 1) * VCH],
                in_=lg)

    ctx.close()


# ---------------- host side ----------------

_PROG_CACHE = {}


def _get_program():
    if "nc" not in _PROG_CACHE:
        _PROG_CACHE["nc"] = build_program()
    return _PROG_CACHE["nc"]


def _tile_kp(w):
    """[K*128, M] -> [128, K*M] pre-tiled layout: out[p, k*M+m] = w[k*128+p, m]."""
    kt = w.shape[0] // 128
    return np.ascontiguousarray(
        w.reshape(kt, 128, -1).transpose(1, 0, 2).reshape(128, -1))


def _col(vv):
    """[K*128] -> [128, K] column layout."""
    return np.ascontiguousarray(vv.reshape(-1, 128).T)


def make_in_maps(input_ids, emb, Wq, Wk, Wv, Wo, W1, b1, W2, b2,
                 ln1_g, ln1_b, ln2_g, ln2_b, lnf_g, lnf_b, head_w, cfg=None):
    c_ = dict(CFG)
    if cfg:
        c_.update(cfg)
    TP = c_["TP"]
    D, V, L, S = c_["D"], c_["V"], c_["L"], c_["S"]
    TS = S // TP
    DF = 4 * D
    DSH, VSH = D // TP, V // TP
    KT = D // 128
    KF = DF // 128
    VCH = 500
    NV = VSH // VCH
    bf = ml_dtypes.bfloat16
    f32 = np.float32
    emb = np.asarray(emb)
    input_ids = np.asarray(input_ids)

    def w2_layout(w):
        # [DF, D] -> [128, KT, KF*128]: out[p, m, k*128+j] = w[k*128+p, m*128+j]
        return np.ascontiguousarray(
            w.reshape(KF, 128, KT, 128).transpose(1, 2, 0, 3)
            .reshape(128, KT, KF * 128))

    in_maps = []
    for c in range(N_CORES):
        g, r = c // TP, c % TP
        x0 = emb[input_ids[g]]                                   # [S, D] f32
        xT = np.ascontiguousarray(x0.T[:, r * TS:(r + 1) * TS]).astype(f32)
        m = {
            "xT0": xT,
            "wq": np.stack([_tile_kp(np.asarray(Wq[l])[:, r * DSH:(r + 1) * DSH])
                            for l in range(L)]).astype(bf),
            "wk": np.stack([_tile_kp(np.asarray(Wk[l])[:, r * DSH:(r + 1) * DSH])
                            for l in range(L)]).astype(bf),
            "wv": np.stack([_tile_kp(np.asarray(Wv[l])[:, r * DSH:(r + 1) * DSH])
                            for l in range(L)]).astype(bf),
            "wo": np.stack([_tile_kp(np.asarray(Wo[l])[r * DSH:(r + 1) * DSH, :])
                            for l in range(L)]).astype(bf),
            "w1": np.stack([_tile_kp(np.asarray(W1[l]))
                            for l in range(L)]).astype(bf),
            "w2": np.stack([w2_layout(np.asarray(W2[l]))
                            for l in range(L)]).astype(bf),
            "b1": np.stack([_col(np.asarray(b1[l])) for l in range(L)]).astype(f32),
            "b2": np.stack([_col(np.asarray(b2[l])) for l in range(L)]).astype(f32),
            "g1": np.stack([_col(np.asarray(ln1_g[l])) for l in range(L)]).astype(f32),
            "be1": np.stack([_col(np.asarray(ln1_b[l])) for l in range(L)]).astype(f32),
            "g2": np.stack([_col(np.asarray(ln2_g[l])) for l in range(L)]).astype(f32),
            "be2": np.stack([_col(np.asarray(ln2_b[l])) for l in range(L)]).astype(f32),
            "gf": _col(np.asarray(lnf_g)).astype(f32),
            "bef": _col(np.asarray(lnf_b)).astype(f32),
            "hw": np.ascontiguousarray(
                np.asarray(head_w)[:, r * VSH:(r + 1) * VSH]
                .reshape(KT, 128, NV, VCH).transpose(1, 2, 0, 3)).astype(bf),
        }
        in_maps.append(m)
    return in_maps


def _assemble(res):
    B, S, V = CFG["B"], CFG["S"], CFG["V"]
    TP = CFG["TP"]
    VSH = V // TP
    out = np.empty((B, S, V), dtype=np.float32)
    for c in range(N_CORES):
        g, r = c // TP, c % TP
        out[g, :, r * VSH:(r + 1) * VSH] = res.results[c]["logits"]
    return out


def kernel(**inputs):
    nc = _get_program()
    in_maps = make_in_maps(**inputs)
    res = run_bass_kernel_spmd(nc, in_maps, list(range(N_CORES)), trace=False)
    return _assemble(res)


def run_traced(**inputs):
    """Like kernel() but with NTFF tracing; returns (out, exec_time_ns)."""
    nc = _get_program()
    in_maps = make_in_maps(**inputs)
    res = run_bass_kernel_spmd(nc, in_maps, list(range(N_CORES)), trace=True)
    return _assemble(res), res.exec_time_ns
